# revision 58
# baseline (speedup 1.0000x reference)
# Trainium2 Bass kernel for the Chebyshev-GCN GRU decoder (gnn_message_passing).
#
# Problem: B=16, N=2048, F=64, K=2 Chebyshev taps, T=8 decode steps.
#   per step: gates = cheb(L, [x, hx]) @ W_gate; r,u = sigmoid(gates)
#             cy = tanh(cheb(L, [x, r*hx]) @ W_upd); hy = u*hx + (1-u)*cy
#             yt = sigmoid(hy @ W_edge)
#
# Strategy (168.2us baseline -> 141.7us; HW rel err 1.32e-2):
#  - Data-parallel over batch: 8 cores x 2 batches each; c = b*64+f = 128
#    partitions for all "transposed"-layout [c, n] tensors.
#  - Big matmuls (L@hx, L@(r*hx)) are fp8e4m3 DoubleRow: stationary = fp8
#    state m-tile pairs [128, 2, 128], moving = fp8 L^T [128, 2, 256]; one
#    instruction contracts K=256 at 0.5 cycles/row.  L is pre-scaled x64 on
#    host, state x16 on device; scales fold into the fp8 gate weights.
#  - Each big matmul runs as FOUR quarter-psum passes (all 8 K-pairs x 512
#    cols, one bank each, one 512-wide DR instruction per K-pair; `start`
#    zeroes a whole psum bank).  The big ring holds 4 banks so
#    all quarters fly at once; mmB additionally emits pairs 0-5 of every
#    quarter before any pair-6/7 tail so the in-order PE queue never blocks
#    early quarters on the last-arriving stationary block.
#  - Natural-layout fp8 stationaries come from PE transposes (bf16 in,
#    bf16 psum out, 53ns per 128x128 tile) instead of DMA XBAR: the DMA
#    path cost 625 (queue) + 448 (xfer) + 900 (sem) per block on the
#    recurrence chain; the PE path is 212 + evac-cast + 100 sem.  The
#    psum->SBUF evac casts double as the x16 fp8 quantization.
#  - r-gate DRs interleave between mmA quarter passes so the sigma_r
#    cascade hides under mmA; r2/r3 psums borrow big-ring banks so the
#    2-bank r ring (shared with the u gates) never gates the cascade.
#    The u path (sigma_u, W = u*hx on Pool, v = 1-u) rides in mmB's
#    shadow; blend is pp = v*cy, hy = W + pp (2 DVE hops after each tanh).
#  - PSUM (8 banks): big ring 4x[128,512] f32 (psA/psB quarters, r2/r3,
#    cand, edge); r/u ring 2x[128,512]; transpose ring 2x[128,1024] bf16.
#  - Pre-roll: lt8 is loaded in column-quarter DMAs (SBUF [128,4,NT,512])
#    ordered xnat8 -> lt8q0 -> xT/wpackb/bpack -> lt8q1..3 -> wpack8, so
#    the precompute big-matmul quarters, the gate constants gc{r,u,c}
#    (DVE stt + bias, emitted u/c before r since step 0 only needs u,c)
#    and step 0 pipeline behind the 11.6us L load.
#  - Output sigmoids land in an SBUF ring, DMA'd out two steps later.
#
# kernel() takes FULL unsharded inputs, returns FULL [T, B, N, F] fp32.

import numpy as np
from contextlib import ExitStack

import concourse.bass as bass
import concourse.tile as tile
from concourse import bacc, mybir
from concourse.bass_utils import run_bass_kernel_spmd

F32 = mybir.dt.float32
BF16 = mybir.dt.bfloat16
FP8 = mybir.dt.float8e4
DR = mybir.MatmulPerfMode.DoubleRow

B, N, F = 16, 2048, 64
T = 8
NCORES = 8
BL = B // NCORES          # batches per core (2)
C = BL * F                # 128 partitions
NT = N // 128             # 16 m-tiles
NBLK = 4                  # n blocks for elementwise/small-mm work
BLK = N // NBLK           # 512
NPAIR = NT // 2           # 8 DoubleRow K-pairs

S_L = 64.0                # L^T fp8 pre-scale (host)
S_H = 16.0                # hx / rh fp8 cast scale (device)
S_C = 4.0                 # lxh8 scale; psum A holds S_L*S_H*lxh
G = 64.0                  # gate/cand PSUM pre-activation scale

W8_PAIRS = ["w8_r", "w8_u", "w8_c"]
WB_NAMES = ["identb", "web", "wx0r", "wx1r", "wx0u", "wx1u", "wxc0", "wxc1"]
B_NAMES = ["bgr", "bgu", "bcc", "bee"]

MUL = mybir.AluOpType.mult
ADD = mybir.AluOpType.add
SUBTRACT = mybir.AluOpType.subtract


def _emit(ctx: ExitStack, tc: tile.TileContext, d):
    nc = tc.nc
    AF = mybir.ActivationFunctionType

    consts = ctx.enter_context(tc.tile_pool(name="consts", bufs=1))
    work = ctx.enter_context(tc.tile_pool(name="work", bufs=3))
    # PSUM (8 banks): big ring 4x[128,512] = 4 banks (all 4 quarter passes of
    # a big matmul can be in flight); r/u shared ring 2x[128,512] (u-gate
    # psums reuse the r banks once sigma_r consumed them); transpose ring
    # 2x[128,1024] bf16 = 2 banks.
    big_ps = ctx.enter_context(tc.tile_pool(name="bigps", bufs=4, space="PSUM"))
    r_ps = ctx.enter_context(tc.tile_pool(name="rps", bufs=2, space="PSUM"))
    u_ps = r_ps
    t_ps = ctx.enter_context(tc.tile_pool(name="tps", bufs=2, space="PSUM"))

    # ---- static loads -------------------------------------------------
    # Order matters: the single DMA queue serializes transfers, and the
    # precompute big-matmul quarter q is gated by xnat8 + lt8 quarter q.
    xnat8 = consts.tile([128, NT, 128], FP8, tag="xnat8")
    try:
        nc.sync.dma_start(xnat8[:], d["xnat8"].rearrange("(a p) c -> p a c", p=128))
    except Exception:
        for mi in range(NT):
            nc.sync.dma_start(xnat8[:, mi, :],
                              d["xnat8"][mi * 128:(mi + 1) * 128, :])
    lt8 = consts.tile([128, 4, NT, BLK], FP8, tag="lt8")
    nc.sync.dma_start(lt8[:, 0, :, :],
                      d["lt8"][0].rearrange("(a p) c -> p a c", p=128))
    xT = consts.tile([128, N], BF16, tag="xT")
    nc.sync.dma_start(xT[:], d["xT"][:, :])
    wpackb = consts.tile([128, len(WB_NAMES) * 128], BF16, tag="wpackb")
    nc.sync.dma_start(wpackb[:], d["wpackb"][:, :])
    wb = {name: wpackb[:, i * 128:(i + 1) * 128]
          for i, name in enumerate(WB_NAMES)}
    bpack = consts.tile([128, len(B_NAMES)], F32, tag="bpack")
    nc.sync.dma_start(bpack[:], d["bpack"][:, :])
    bias = {name: bpack[:, j:j + 1] for j, name in enumerate(B_NAMES)}
    for q in range(1, 4):
        nc.sync.dma_start(lt8[:, q, :, :],
                          d["lt8"][q].rearrange("(a p) c -> p a c", p=128))
    wpack8 = consts.tile([128, 6, 128], FP8, tag="wpack8")
    nc.sync.dma_start(wpack8[:], d["wpack8"][:, :].rearrange("p (i m) -> p i m", i=6))
    w8 = {name: wpack8[:, 2 * i:2 * i + 2, :] for i, name in enumerate(W8_PAIRS)}

    # ---- persistent step buffers --------------------------------------
    hxbuf = [consts.tile([128, N], BF16, tag=f"hxT{i}", name=f"hxT{i}")
             for i in range(2)]
    ruT = consts.tile([128, 2, N], BF16, tag="ruT")   # r plane | u plane
    cyT = consts.tile([128, N], BF16, tag="cyT")
    rhT = consts.tile([128, N], BF16, tag="rhT")
    s8hy = consts.tile([128, NT, 128], FP8, tag="s8hy")
    s8rh = consts.tile([128, NT, 128], FP8, tag="s8rh")
    rupack = consts.tile([128, 2, N], FP8, tag="rupack")      # hxT8 | lxh8
    candpack = consts.tile([128, 2, N], FP8, tag="candpack")  # rhT8 | lrh8
    WT = consts.tile([128, N], BF16, tag="WT")
    vT = consts.tile([128, N], BF16, tag="vT")
    gcr = consts.tile([128, N], BF16, tag="gcr")
    gcu = consts.tile([128, N], BF16, tag="gcu")
    gcc = consts.tile([128, N], BF16, tag="gcc")
    LxT = consts.tile([128, N], BF16, tag="LxT")
    ytbuf = consts.tile([128, T * N], BF16, tag="ytbuf")

    def nb(ap, blk):
        return ap[:, blk * BLK:(blk + 1) * BLK]

    def big_mm_quarter(stat8, ps, q):
        """One psum quarter (cols [512q, 512q+512)) of the DoubleRow big
        matmul: all 8 K-pairs, 2x256-col instructions each."""
        for p in range(NPAIR):
            nc.tensor.matmul(
                ps[:], stat8[:, 2 * p:2 * p + 2, :],
                lt8[:, q, 2 * p:2 * p + 2, :],
                start=(p == 0), stop=(p == NPAIR - 1),
                perf_mode=DR, skip_group_check=True)

    def big_mm_pairs(stat8, ps, q, pairs, start_p, stop_p):
        for p in pairs:
            nc.tensor.matmul(
                ps[:], stat8[:, 2 * p:2 * p + 2, :],
                lt8[:, q, 2 * p:2 * p + 2, :],
                start=(p == start_p), stop=(p == stop_p),
                perf_mode=DR, skip_group_check=True)

    def gate_ident(ps, gc, blk):
        nc.tensor.matmul(ps[:], wb["identb"], nb(gc, blk),
                         start=True, stop=False, skip_group_check=True)

    def gate_dr(ps, wpair, pack, blk):
        nc.tensor.matmul(
            ps[:], wpair, pack[:, :, blk * BLK:(blk + 1) * BLK],
            start=False, stop=True,
            perf_mode=DR, skip_group_check=True)

    def pe_transpose(tr, half, src_row, blk):
        """Transpose block blk of bf16 row src_row into tr cols
        [512*half, 512*half+512) as 4 PE 128x128 transposes."""
        for j in range(4):
            nc.tensor.matmul(
                tr[:, half * BLK + j * 128:half * BLK + (j + 1) * 128],
                src_row[:, blk * BLK + j * 128:blk * BLK + (j + 1) * 128],
                wb["identb"], is_transpose=True,
                skip_group_check=True).annotate(f'T{blk}_{j}')

    def cast_fp8(eng, dst, src, scale):
        if eng is nc.scalar:
            return eng.mul(dst, src, float(scale))
        return eng.tensor_scalar_mul(dst, src, float(scale))

    SC_LXH = S_C / (S_L * S_H)

    # =========== precompute ============================================
    for q in range(4):
        pq = big_ps.tile([128, BLK], F32, tag="big", name=f"pre{q}")
        big_mm_quarter(xnat8, pq, q)
        cast_fp8(nc.scalar, LxT[:, q * BLK:(q + 1) * BLK], pq[:],
                 1.0 / (S_L * S_H))
    for blk in range(NBLK):
        for pool, tg, wa, wc_, dst, bs in (
                (u_ps, "r", "wx0u", "wx1u", gcu, "bgu"),
                (r_ps, "r", "wxc0", "wxc1", gcc, "bcc"),
                (r_ps, "r", "wx0r", "wx1r", gcr, "bgr")):
            ps = pool.tile([128, BLK], F32, tag=tg, name="cps")
            nc.tensor.matmul(ps[:], wb[wa], nb(xT, blk), start=True, stop=False,
                             skip_group_check=True)
            nc.tensor.matmul(ps[:], wb[wc_], nb(LxT, blk), start=False, stop=True,
                             skip_group_check=True)
            nc.vector.scalar_tensor_tensor(
                nb(dst, blk), ps[:], 1.0,
                bias[bs][:].broadcast_to([128, BLK]), op0=MUL, op1=ADD)

    # =========== step 0 (hx == 0) ======================================
    hyT = hxbuf[1]
    uT0 = ruT[:, 1, :]
    for blk in range(NBLK):
        nc.scalar.activation(nb(uT0, blk), nb(gcu, blk), AF.Sigmoid,
                             scale=1.0 / G)
        nc.scalar.activation(nb(cyT, blk), nb(gcc, blk), AF.Tanh,
                             scale=1.0 / G)
        e = work.tile([128, BLK], BF16, tag="tmp", name="e0")
        nc.vector.tensor_mul(e[:], nb(uT0, blk), nb(cyT, blk))
        nc.vector.tensor_sub(nb(hyT, blk), nb(cyT, blk), e[:])
    tr0 = [t_ps.tile([128, 2 * BLK], BF16, tag="tr", name=f"tr0{h}")
           for h in range(2)]
    for blk in range(NBLK):
        pe_transpose(tr0[blk // 2], blk % 2, hyT, blk)
    for blk in range(NBLK):
        eng = (nc.vector, nc.scalar, nc.vector, nc.scalar)[blk]
        cast_fp8(eng, s8hy[:, 4 * blk:4 * (blk + 1), :],
                 tr0[blk // 2][:, (blk % 2) * BLK:(blk % 2 + 1) * BLK], S_H)
    for blk in range(NBLK):
        cast_fp8(nc.gpsimd, rupack[:, 0, blk * BLK:(blk + 1) * BLK],
                 nb(hyT, blk), S_H)
    with tc.high_priority(offset=-400):
        for blk in range(NBLK):
            ep = big_ps.tile([128, BLK], F32, tag="big", name=f"edg0{blk}")
            nc.tensor.matmul(ep[:], wb["web"], nb(hyT, blk), start=True,
                             stop=True, skip_group_check=True)
            nc.scalar.activation(ytbuf[:, blk * BLK:(blk + 1) * BLK], ep[:],
                                 AF.Sigmoid, bias=bias["bee"][:])

    # =========== steps 1..T-1 ==========================================
    # Per step, the critical chain is:
    #   mmA q -> lxh cast -> rDR -> sigma_r -> rh mul -> PE transpose ->
    #   s8rh evac-cast -> mmB -> lrh cast -> candDR -> tanh -> d/m/hy ->
    #   PE transpose -> s8hy evac-cast -> mmA' ...
    # r-gate DRs are interleaved between mmA quarter passes so the sigma_r
    # cascade hides under mmA; r2/r3 psums come from the big ring so the
    # r ring (which also serves the u gates) never gates the cascade.
    rps = [None] * NBLK
    ups = [None] * NBLK

    def alloc_r(blk, pool):
        rps[blk] = pool.tile([128, BLK], F32, tag=("r" if pool is r_ps else "big"),
                             name="rps")
        gate_ident(rps[blk], gcr, blk)

    def alloc_u(blk):
        ups[blk] = u_ps.tile([128, BLK], F32, tag="r", name="ups")
        gate_ident(ups[blk], gcu, blk)

    rT = ruT[:, 0, :]
    uT = ruT[:, 1, :]

    for t in range(1, T):
        hxT, hyT = hxbuf[t % 2], hxbuf[(t + 1) % 2]
        # --- phase A: mmA quarters with rDRs interleaved ---------------
        psA = [None] * 4
        for q in range(4):
            psA[q] = big_ps.tile([128, BLK], F32, tag="big", name=f"psA{q}")
            big_mm_quarter(s8hy, psA[q], q)
            cast_fp8(nc.scalar if q == 0 else nc.vector,
                     rupack[:, 1, q * BLK:(q + 1) * BLK],
                     psA[q][:], SC_LXH).annotate(f'lxh{q}')
            if q == 0:
                alloc_r(0, r_ps)
                alloc_r(1, r_ps)
            if q in (1, 2):
                gate_dr(rps[q - 1], w8["w8_r"], rupack, q - 1)
                nc.scalar.activation(nb(rT, q - 1), rps[q - 1][:], AF.Sigmoid,
                                     scale=1.0 / G).annotate(f'sigr{q - 1}')
            elif q == 3:
                alloc_r(2, big_ps)
                gate_dr(rps[2], w8["w8_r"], rupack, 2)
                nc.scalar.activation(nb(rT, 2), rps[2][:], AF.Sigmoid,
                                     scale=1.0 / G).annotate('sigr2')
        alloc_r(3, big_ps)
        gate_dr(rps[3], w8["w8_r"], rupack, 3)
        nc.scalar.activation(nb(rT, 3), rps[3][:], AF.Sigmoid,
                             scale=1.0 / G).annotate('sigr3')
        # rh mul -> PE transpose -> evac (Act for blks 0/1, DVE for 2/3)
        trh = [t_ps.tile([128, 2 * BLK], BF16, tag="tr", name=f"trh{h}")
               for h in range(2)]
        for blk in range(NBLK):
            nc.vector.tensor_mul(nb(rhT, blk), nb(rT, blk),
                                 nb(hxT, blk)).annotate(f'mul{blk}')
            pe_transpose(trh[blk // 2], blk % 2, rhT, blk)
            cast_fp8(nc.gpsimd, candpack[:, 0, blk * BLK:(blk + 1) * BLK],
                     nb(rhT, blk), S_H).annotate(f'cp0_{blk}')
            eng = (nc.scalar, nc.scalar, nc.vector, nc.vector)[blk]
            cast_fp8(eng, s8rh[:, 4 * blk:4 * (blk + 1), :],
                     trh[blk // 2][:, (blk % 2) * BLK:(blk % 2 + 1) * BLK],
                     S_H).annotate(f'evR{blk}')
        # --- phase B: mmB quarters; u gates + lrh casts in the shadow --
        psB = [None] * 4
        for q in range(4):
            psB[q] = big_ps.tile([128, BLK], F32, tag="big", name=f"psB{q}")
            big_mm_pairs(s8rh, psB[q], q, range(6), 0, NPAIR - 1)
        for q in range(4):
            big_mm_pairs(s8rh, psB[q], q, (6, 7), 0, NPAIR - 1)
            cast_fp8(nc.vector, candpack[:, 1, q * BLK:(q + 1) * BLK],
                     psB[q][:], SC_LXH).annotate(f'lrh{q}')
            if q >= 2:
                blk = q - 2
                alloc_u(blk)
                gate_dr(ups[blk], w8["w8_u"], rupack, blk)
                nc.scalar.activation(nb(uT, blk), ups[blk][:], AF.Sigmoid,
                                     scale=1.0 / G)
        for blk in (2, 3):
            alloc_u(blk)
            gate_dr(ups[blk], w8["w8_u"], rupack, blk)
            nc.scalar.activation(nb(uT, blk), ups[blk][:], AF.Sigmoid,
                                 scale=1.0 / G)
        for blk in range(NBLK):
            nc.gpsimd.tensor_mul(nb(WT, blk), nb(uT, blk),
                                 nb(hxT, blk)).annotate(f'W{blk}')
            nc.vector.tensor_scalar(nb(vT, blk), nb(uT, blk), -1.0, 1.0,
                                    op0=MUL, op1=ADD).annotate(f'v{blk}')
        # --- phase B2: cand -> tanh -> blend (hy = cy + u*(hx-cy)) -----
        candps = [None] * 4
        for q in range(4):
            cp = big_ps.tile([128, BLK], F32, tag="big", name=f"cand{q}")
            candps[q] = cp
            gate_ident(cp, gcc, q)
            gate_dr(cp, w8["w8_c"], candpack, q)
        for q in range(4):
            nc.scalar.activation(nb(cyT, q), candps[q][:], AF.Tanh,
                                 scale=1.0 / G).annotate(f'tanh{q}')
        thy = [t_ps.tile([128, 2 * BLK], BF16, tag="tr", name=f"thy{h}")
               for h in range(2)] if t < T - 1 else None

        def blend(blk):
            pp = work.tile([128, BLK], BF16, tag="tmp", name="pp")
            nc.vector.tensor_mul(pp[:], nb(vT, blk), nb(cyT, blk)).annotate(f'p{blk}')
            nc.vector.tensor_add(nb(hyT, blk), nb(WT, blk), pp[:]).annotate(f'hy{blk}')

        def hyexp(blk):
            if t < T - 1:
                pe_transpose(thy[blk // 2], blk % 2, hyT, blk)

        def hyevac(blk):
            if t >= T - 1:
                return
            eng = (nc.scalar, nc.vector, nc.scalar, nc.vector)[blk]
            cast_fp8(eng, s8hy[:, 4 * blk:4 * (blk + 1), :],
                     thy[blk // 2][:, (blk % 2) * BLK:(blk % 2 + 1) * BLK],
                     S_H).annotate(f'evH{blk}')

        for blk in range(NBLK):
            blend(blk)
            hyexp(blk)
            hyevac(blk)
        if t < T - 1:
            cast_fp8(nc.gpsimd, rupack[:, 0, :], hyT[:], S_H)
        with tc.high_priority(offset=-400):
            for blk in range(NBLK):
                ep = big_ps.tile([128, BLK], F32, tag="big", name=f"edge{blk}")
                nc.tensor.matmul(ep[:], wb["web"], nb(hyT, blk), start=True,
                                 stop=True, skip_group_check=True)
                nc.scalar.activation(
                    ytbuf[:, t * N + blk * BLK:t * N + (blk + 1) * BLK],
                    ep[:], AF.Sigmoid, bias=bias["bee"][:])
        if t >= 2:
            nc.sync.dma_start(d["out"][t - 2, :, :], ytbuf[:, (t - 2) * N:(t - 1) * N])
        if t == T - 1:
            # out[T-2] has been ready since the previous step's sigmoids;
            # out[T-1] goes per block, each DMA chasing its own sigmoid, so
            # the final transfer tail is one [128,512] DMA, not a full row.
            nc.sync.dma_start(d["out"][T - 2, :, :],
                              ytbuf[:, (T - 2) * N:(T - 1) * N])
            for blk in range(NBLK):
                nc.sync.dma_start(
                    d["out"][T - 1, :, blk * BLK:(blk + 1) * BLK],
                    ytbuf[:, (T - 1) * N + blk * BLK:(T - 1) * N + (blk + 1) * BLK])


_BUILT = {}


def _build():
    if "nc" in _BUILT:
        return _BUILT["nc"]
    nc = bacc.Bacc("TRN2", target_bir_lowering=False, debug=False)
    d = {}
    d["lt8"] = nc.dram_tensor("lt8", [4, N, BLK], FP8,
                              kind="ExternalInput").ap()
    d["xnat8"] = nc.dram_tensor("xnat8", [N, C], FP8, kind="ExternalInput").ap()
    d["xT"] = nc.dram_tensor("xT", [C, N], BF16, kind="ExternalInput").ap()
    d["wpack8"] = nc.dram_tensor("wpack8", [128, 6 * 128], FP8,
                                 kind="ExternalInput").ap()
    d["wpackb"] = nc.dram_tensor("wpackb", [128, len(WB_NAMES) * 128], BF16,
                                 kind="ExternalInput").ap()
    d["bpack"] = nc.dram_tensor("bpack", [128, len(B_NAMES)], F32,
                                kind="ExternalInput").ap()
    d["out"] = nc.dram_tensor("out", [T, C, N], BF16, kind="ExternalOutput").ap()

    with tile.TileContext(nc) as tc, ExitStack() as ctx:
        _emit(ctx, tc, d)
    nc.compile()
    _BUILT["nc"] = nc
    return nc


def _bd(m):
    """[64,64] -> block-diagonal [128,128] (two independent batches)."""
    z = np.zeros((128, 128), np.float32)
    z[:64, :64] = m
    z[64:, 64:] = m
    return z


def _q8(a):
    import ml_dtypes
    e4 = getattr(ml_dtypes, "float8_e4m3fn", None) or ml_dtypes.float8_e4m3
    return np.clip(np.asarray(a, np.float32), -240.0, 240.0).astype(e4)


def _bf(a):
    import ml_dtypes
    return np.asarray(a, np.float32).astype(ml_dtypes.bfloat16)


def make_in_maps(inputs_edge, L_tilde, W_gate, b_gate, W_upd, b_upd,
                 W_edge, b_edge):
    """Host-side layout transforms + quantization + per-core sharding."""
    x = np.asarray(inputs_edge, np.float32)
    L = np.asarray(L_tilde, np.float32)
    Wg0, Wg1 = np.asarray(W_gate[0], np.float32), np.asarray(W_gate[1], np.float32)
    Wu0, Wu1 = np.asarray(W_upd[0], np.float32), np.asarray(W_upd[1], np.float32)
    We = np.asarray(W_edge, np.float32)
    bg = np.asarray(b_gate, np.float32)
    bu = np.asarray(b_upd, np.float32)
    be = np.asarray(b_edge, np.float32)

    # fp8 DR weight pairs, scales folded:
    #   slot0 (vs hxT8 = S_H*hx):   (G/S_H) * wh0
    #   slot1 (vs lxh8 = S_C*lxh):  (G/S_C) * wh1
    s0, s1 = G / S_H, G / S_C
    wpack8 = np.concatenate([
        _bd(s0 * Wg0[F:, :F]), _bd(s1 * Wg1[F:, :F]),      # r
        _bd(s0 * Wg0[F:, F:]), _bd(s1 * Wg1[F:, F:]),      # u
        _bd(s0 * Wu0[F:]), _bd(s1 * Wu1[F:]),              # cand
    ], axis=1)
    wpackb = np.concatenate([
        np.eye(128, dtype=np.float32), _bd(We),
        _bd(G * Wg0[:F, :F]), _bd(G * Wg1[:F, :F]),
        _bd(G * Wg0[:F, F:]), _bd(G * Wg1[:F, F:]),
        _bd(G * Wu0[:F]), _bd(G * Wu1[:F]),
    ], axis=1)
    bpack = np.stack([G * np.tile(bg[:F], 2), G * np.tile(bg[F:], 2),
                      G * np.tile(bu, 2), np.tile(be, 2)], axis=1)
    shared = {
        "lt8": np.ascontiguousarray(
            _q8(S_L * L.T).reshape(N, 4, BLK).transpose(1, 0, 2)),
        "wpack8": np.ascontiguousarray(_q8(wpack8)),
        "wpackb": np.ascontiguousarray(_bf(wpackb)),
        "bpack": np.ascontiguousarray(bpack.astype(np.float32)),
    }
    in_maps = []
    for core in range(NCORES):
        xs = x[core * BL:(core + 1) * BL]                    # [BL, N, F]
        m = dict(shared)
        m["xnat8"] = np.ascontiguousarray(
            _q8(S_H * xs.transpose(1, 0, 2).reshape(N, C)))
        m["xT"] = np.ascontiguousarray(
            _bf(xs.transpose(0, 2, 1).reshape(C, N)))
        in_maps.append(m)
    return in_maps


def unshard(core_outs):
    """[NCORES][T, C, N] (bf16) -> [T, B, N, F] fp32"""
    arr = np.stack([np.asarray(o, np.float32) for o in core_outs])
    return np.ascontiguousarray(
        arr.reshape(NCORES, T, BL, F, N)
           .transpose(1, 0, 2, 4, 3)
           .reshape(T, B, N, F).astype(np.float32))


def run(in_maps, **kw):
    nc = _build()
    return run_bass_kernel_spmd(nc, in_maps, list(range(NCORES)), **kw)


def kernel(inputs_edge, L_tilde, W_gate, b_gate, W_upd, b_upd, W_edge, b_edge):
    in_maps = make_in_maps(inputs_edge, L_tilde, W_gate, b_gate,
                           W_upd, b_upd, W_edge, b_edge)
    res = run(in_maps)
    return unshard([res.results[c]["out"] for c in range(NCORES)])


# revision 59
# speedup vs baseline: 1.0001x; 1.0001x over previous
# Trainium2 Bass kernel for the Chebyshev-GCN GRU decoder (gnn_message_passing).
#
# Problem: B=16, N=2048, F=64, K=2 Chebyshev taps, T=8 decode steps.
#   per step: gates = cheb(L, [x, hx]) @ W_gate; r,u = sigmoid(gates)
#             cy = tanh(cheb(L, [x, r*hx]) @ W_upd); hy = u*hx + (1-u)*cy
#             yt = sigmoid(hy @ W_edge)
#
# Strategy (168.2us baseline -> 141.7us; HW rel err 1.32e-2):
#  - Data-parallel over batch: 8 cores x 2 batches each; c = b*64+f = 128
#    partitions for all "transposed"-layout [c, n] tensors.
#  - Big matmuls (L@hx, L@(r*hx)) are fp8e4m3 DoubleRow: stationary = fp8
#    state m-tile pairs [128, 2, 128], moving = fp8 L^T [128, 2, 256]; one
#    instruction contracts K=256 at 0.5 cycles/row.  L is pre-scaled x64 on
#    host, state x16 on device; scales fold into the fp8 gate weights.
#  - Each big matmul runs as FOUR quarter-psum passes (all 8 K-pairs x 512
#    cols, one bank each, one 512-wide DR instruction per K-pair; `start`
#    zeroes a whole psum bank).  The big ring holds 4 banks so
#    all quarters fly at once; mmB additionally emits pairs 0-5 of every
#    quarter before any pair-6/7 tail so the in-order PE queue never blocks
#    early quarters on the last-arriving stationary block.
#  - Natural-layout fp8 stationaries come from PE transposes (bf16 in,
#    bf16 psum out, 53ns per 128x128 tile) instead of DMA XBAR: the DMA
#    path cost 625 (queue) + 448 (xfer) + 900 (sem) per block on the
#    recurrence chain; the PE path is 212 + evac-cast + 100 sem.  The
#    psum->SBUF evac casts double as the x16 fp8 quantization.
#  - r-gate DRs interleave between mmA quarter passes so the sigma_r
#    cascade hides under mmA; r2/r3 psums borrow big-ring banks so the
#    2-bank r ring (shared with the u gates) never gates the cascade.
#    The u path (sigma_u, W = u*hx on Pool, v = 1-u) rides in mmB's
#    shadow; blend is pp = v*cy, hy = W + pp (2 DVE hops after each tanh).
#  - PSUM (8 banks): big ring 4x[128,512] f32 (psA/psB quarters, r2/r3,
#    cand, edge); r/u ring 2x[128,512]; transpose ring 2x[128,1024] bf16.
#  - Pre-roll: lt8 is loaded in column-quarter DMAs (SBUF [128,4,NT,512])
#    ordered xnat8 -> lt8q0 -> xT/wpackb/bpack -> lt8q1..3 -> wpack8, so
#    the precompute big-matmul quarters, the gate constants gc{r,u,c}
#    (DVE stt + bias, emitted u/c before r since step 0 only needs u,c)
#    and step 0 pipeline behind the 11.6us L load.
#  - Output sigmoids land in an SBUF ring, DMA'd out two steps later.
#
# kernel() takes FULL unsharded inputs, returns FULL [T, B, N, F] fp32.

import numpy as np
from contextlib import ExitStack

import concourse.bass as bass
import concourse.tile as tile
from concourse import bacc, mybir
from concourse.bass_utils import run_bass_kernel_spmd

F32 = mybir.dt.float32
BF16 = mybir.dt.bfloat16
FP8 = mybir.dt.float8e4
DR = mybir.MatmulPerfMode.DoubleRow

B, N, F = 16, 2048, 64
T = 8
NCORES = 8
BL = B // NCORES          # batches per core (2)
C = BL * F                # 128 partitions
NT = N // 128             # 16 m-tiles
NBLK = 4                  # n blocks for elementwise/small-mm work
BLK = N // NBLK           # 512
NPAIR = NT // 2           # 8 DoubleRow K-pairs

S_L = 64.0                # L^T fp8 pre-scale (host)
S_H = 16.0                # hx / rh fp8 cast scale (device)
S_C = 4.0                 # lxh8 scale; psum A holds S_L*S_H*lxh
G = 64.0                  # gate/cand PSUM pre-activation scale

W8_PAIRS = ["w8_r", "w8_u", "w8_c"]
WB_NAMES = ["identb", "web", "wx0r", "wx1r", "wx0u", "wx1u", "wxc0", "wxc1"]
B_NAMES = ["bgr", "bgu", "bcc", "bee"]

MUL = mybir.AluOpType.mult
ADD = mybir.AluOpType.add
SUBTRACT = mybir.AluOpType.subtract


def _emit(ctx: ExitStack, tc: tile.TileContext, d):
    nc = tc.nc
    AF = mybir.ActivationFunctionType

    consts = ctx.enter_context(tc.tile_pool(name="consts", bufs=1))
    work = ctx.enter_context(tc.tile_pool(name="work", bufs=3))
    # PSUM (8 banks): big ring 4x[128,512] = 4 banks (all 4 quarter passes of
    # a big matmul can be in flight); r/u shared ring 2x[128,512] (u-gate
    # psums reuse the r banks once sigma_r consumed them); transpose ring
    # 2x[128,1024] bf16 = 2 banks.
    big_ps = ctx.enter_context(tc.tile_pool(name="bigps", bufs=4, space="PSUM"))
    r_ps = ctx.enter_context(tc.tile_pool(name="rps", bufs=2, space="PSUM"))
    u_ps = r_ps
    t_ps = ctx.enter_context(tc.tile_pool(name="tps", bufs=2, space="PSUM"))

    # ---- static loads -------------------------------------------------
    # Order matters: the single DMA queue serializes transfers, and the
    # precompute big-matmul quarter q is gated by xnat8 + lt8 quarter q.
    xnat8 = consts.tile([128, NT, 128], FP8, tag="xnat8")
    try:
        nc.sync.dma_start(xnat8[:], d["xnat8"].rearrange("(a p) c -> p a c", p=128))
    except Exception:
        for mi in range(NT):
            nc.sync.dma_start(xnat8[:, mi, :],
                              d["xnat8"][mi * 128:(mi + 1) * 128, :])
    lt8 = consts.tile([128, 4, NT, BLK], FP8, tag="lt8")
    nc.sync.dma_start(lt8[:, 0, :, :],
                      d["lt8"][0].rearrange("(a p) c -> p a c", p=128))
    xT = consts.tile([128, N], BF16, tag="xT")
    nc.sync.dma_start(xT[:], d["xT"][:, :])
    wpackb = consts.tile([128, len(WB_NAMES) * 128], BF16, tag="wpackb")
    nc.sync.dma_start(wpackb[:], d["wpackb"][:, :])
    wb = {name: wpackb[:, i * 128:(i + 1) * 128]
          for i, name in enumerate(WB_NAMES)}
    bpack = consts.tile([128, len(B_NAMES)], F32, tag="bpack")
    nc.sync.dma_start(bpack[:], d["bpack"][:, :])
    bias = {name: bpack[:, j:j + 1] for j, name in enumerate(B_NAMES)}
    for q in range(1, 4):
        nc.sync.dma_start(lt8[:, q, :, :],
                          d["lt8"][q].rearrange("(a p) c -> p a c", p=128))
    wpack8 = consts.tile([128, 6, 128], FP8, tag="wpack8")
    nc.sync.dma_start(wpack8[:], d["wpack8"][:, :].rearrange("p (i m) -> p i m", i=6))
    w8 = {name: wpack8[:, 2 * i:2 * i + 2, :] for i, name in enumerate(W8_PAIRS)}

    # ---- persistent step buffers --------------------------------------
    hxbuf = [consts.tile([128, N], BF16, tag=f"hxT{i}", name=f"hxT{i}")
             for i in range(2)]
    ruT = consts.tile([128, 2, N], BF16, tag="ruT")   # r plane | u plane
    cyT = consts.tile([128, N], BF16, tag="cyT")
    rhT = consts.tile([128, N], BF16, tag="rhT")
    s8hy = consts.tile([128, NT, 128], FP8, tag="s8hy")
    s8rh = consts.tile([128, NT, 128], FP8, tag="s8rh")
    rupack = consts.tile([128, 2, N], FP8, tag="rupack")      # hxT8 | lxh8
    candpack = consts.tile([128, 2, N], FP8, tag="candpack")  # rhT8 | lrh8
    WT = consts.tile([128, N], BF16, tag="WT")
    vT = consts.tile([128, N], BF16, tag="vT")
    gcr = consts.tile([128, N], BF16, tag="gcr")
    gcu = consts.tile([128, N], BF16, tag="gcu")
    gcc = consts.tile([128, N], BF16, tag="gcc")
    LxT = consts.tile([128, N], BF16, tag="LxT")
    ytbuf = consts.tile([128, T * N], BF16, tag="ytbuf")

    def nb(ap, blk):
        return ap[:, blk * BLK:(blk + 1) * BLK]

    def big_mm_quarter(stat8, ps, q):
        """One psum quarter (cols [512q, 512q+512)) of the DoubleRow big
        matmul: all 8 K-pairs, 2x256-col instructions each."""
        for p in range(NPAIR):
            nc.tensor.matmul(
                ps[:], stat8[:, 2 * p:2 * p + 2, :],
                lt8[:, q, 2 * p:2 * p + 2, :],
                start=(p == 0), stop=(p == NPAIR - 1),
                perf_mode=DR, skip_group_check=True)

    def big_mm_pairs(stat8, ps, q, pairs, start_p, stop_p):
        for p in pairs:
            nc.tensor.matmul(
                ps[:], stat8[:, 2 * p:2 * p + 2, :],
                lt8[:, q, 2 * p:2 * p + 2, :],
                start=(p == start_p), stop=(p == stop_p),
                perf_mode=DR, skip_group_check=True)

    def gate_ident(ps, gc, blk):
        nc.tensor.matmul(ps[:], wb["identb"], nb(gc, blk),
                         start=True, stop=False, skip_group_check=True)

    def gate_dr(ps, wpair, pack, blk):
        nc.tensor.matmul(
            ps[:], wpair, pack[:, :, blk * BLK:(blk + 1) * BLK],
            start=False, stop=True,
            perf_mode=DR, skip_group_check=True)

    def pe_transpose(tr, half, src_row, blk):
        """Transpose block blk of bf16 row src_row into tr cols
        [512*half, 512*half+512) as 4 PE 128x128 transposes."""
        for j in range(4):
            nc.tensor.matmul(
                tr[:, half * BLK + j * 128:half * BLK + (j + 1) * 128],
                src_row[:, blk * BLK + j * 128:blk * BLK + (j + 1) * 128],
                wb["identb"], is_transpose=True,
                skip_group_check=True).annotate(f'T{blk}_{j}')

    def cast_fp8(eng, dst, src, scale):
        if eng is nc.scalar:
            return eng.mul(dst, src, float(scale))
        return eng.tensor_scalar_mul(dst, src, float(scale))

    SC_LXH = S_C / (S_L * S_H)

    # =========== precompute ============================================
    for q in range(4):
        pq = big_ps.tile([128, BLK], F32, tag="big", name=f"pre{q}")
        big_mm_quarter(xnat8, pq, q)
        cast_fp8(nc.scalar, LxT[:, q * BLK:(q + 1) * BLK], pq[:],
                 1.0 / (S_L * S_H))
    for blk in range(NBLK):
        for pool, tg, wa, wc_, dst, bs in (
                (u_ps, "r", "wx0u", "wx1u", gcu, "bgu"),
                (r_ps, "r", "wxc0", "wxc1", gcc, "bcc"),
                (r_ps, "r", "wx0r", "wx1r", gcr, "bgr")):
            ps = pool.tile([128, BLK], F32, tag=tg, name="cps")
            nc.tensor.matmul(ps[:], wb[wa], nb(xT, blk), start=True, stop=False,
                             skip_group_check=True)
            nc.tensor.matmul(ps[:], wb[wc_], nb(LxT, blk), start=False, stop=True,
                             skip_group_check=True)
            nc.vector.scalar_tensor_tensor(
                nb(dst, blk), ps[:], 1.0,
                bias[bs][:].broadcast_to([128, BLK]), op0=MUL, op1=ADD)

    # =========== step 0 (hx == 0) ======================================
    hyT = hxbuf[1]
    uT0 = ruT[:, 1, :]
    for blk in range(NBLK):
        nc.scalar.activation(nb(uT0, blk), nb(gcu, blk), AF.Sigmoid,
                             scale=1.0 / G)
        nc.scalar.activation(nb(cyT, blk), nb(gcc, blk), AF.Tanh,
                             scale=1.0 / G)
        e = work.tile([128, BLK], BF16, tag="tmp", name="e0")
        nc.vector.tensor_mul(e[:], nb(uT0, blk), nb(cyT, blk))
        nc.vector.tensor_sub(nb(hyT, blk), nb(cyT, blk), e[:])
    tr0 = [t_ps.tile([128, 2 * BLK], BF16, tag="tr", name=f"tr0{h}")
           for h in range(2)]
    for blk in range(NBLK):
        pe_transpose(tr0[blk // 2], blk % 2, hyT, blk)
    for blk in range(NBLK):
        eng = (nc.vector, nc.scalar, nc.vector, nc.scalar)[blk]
        cast_fp8(eng, s8hy[:, 4 * blk:4 * (blk + 1), :],
                 tr0[blk // 2][:, (blk % 2) * BLK:(blk % 2 + 1) * BLK], S_H)
    for blk in range(NBLK):
        cast_fp8(nc.gpsimd, rupack[:, 0, blk * BLK:(blk + 1) * BLK],
                 nb(hyT, blk), S_H)
    with tc.high_priority(offset=-400):
        for blk in range(NBLK):
            ep = big_ps.tile([128, BLK], F32, tag="big", name=f"edg0{blk}")
            nc.tensor.matmul(ep[:], wb["web"], nb(hyT, blk), start=True,
                             stop=True, skip_group_check=True)
            nc.scalar.activation(ytbuf[:, blk * BLK:(blk + 1) * BLK], ep[:],
                                 AF.Sigmoid, bias=bias["bee"][:])

    # =========== steps 1..T-1 ==========================================
    # Per step, the critical chain is:
    #   mmA q -> lxh cast -> rDR -> sigma_r -> rh mul -> PE transpose ->
    #   s8rh evac-cast -> mmB -> lrh cast -> candDR -> tanh -> d/m/hy ->
    #   PE transpose -> s8hy evac-cast -> mmA' ...
    # r-gate DRs are interleaved between mmA quarter passes so the sigma_r
    # cascade hides under mmA; r2/r3 psums come from the big ring so the
    # r ring (which also serves the u gates) never gates the cascade.
    rps = [None] * NBLK
    ups = [None] * NBLK

    def alloc_r(blk, pool):
        rps[blk] = pool.tile([128, BLK], F32, tag=("r" if pool is r_ps else "big"),
                             name="rps")
        gate_ident(rps[blk], gcr, blk)

    def alloc_u(blk):
        ups[blk] = u_ps.tile([128, BLK], F32, tag="r", name="ups")
        gate_ident(ups[blk], gcu, blk)

    rT = ruT[:, 0, :]
    uT = ruT[:, 1, :]

    for t in range(1, T):
        hxT, hyT = hxbuf[t % 2], hxbuf[(t + 1) % 2]
        # --- phase A: mmA quarters with rDRs interleaved ---------------
        psA = [None] * 4
        for q in range(4):
            psA[q] = big_ps.tile([128, BLK], F32, tag="big", name=f"psA{q}")
            big_mm_quarter(s8hy, psA[q], q)
            cast_fp8(nc.scalar if q == 1 else nc.vector,
                     rupack[:, 1, q * BLK:(q + 1) * BLK],
                     psA[q][:], SC_LXH).annotate(f'lxh{q}')
            if q == 0:
                alloc_r(0, r_ps)
                alloc_r(1, r_ps)
            if q in (1, 2):
                gate_dr(rps[q - 1], w8["w8_r"], rupack, q - 1)
                nc.scalar.activation(nb(rT, q - 1), rps[q - 1][:], AF.Sigmoid,
                                     scale=1.0 / G).annotate(f'sigr{q - 1}')
            elif q == 3:
                alloc_r(2, big_ps)
                gate_dr(rps[2], w8["w8_r"], rupack, 2)
                nc.scalar.activation(nb(rT, 2), rps[2][:], AF.Sigmoid,
                                     scale=1.0 / G).annotate('sigr2')
        alloc_r(3, big_ps)
        gate_dr(rps[3], w8["w8_r"], rupack, 3)
        nc.scalar.activation(nb(rT, 3), rps[3][:], AF.Sigmoid,
                             scale=1.0 / G).annotate('sigr3')
        # rh mul -> PE transpose -> evac (Act for blks 0/1, DVE for 2/3)
        trh = [t_ps.tile([128, 2 * BLK], BF16, tag="tr", name=f"trh{h}")
               for h in range(2)]
        for blk in range(NBLK):
            nc.vector.tensor_mul(nb(rhT, blk), nb(rT, blk),
                                 nb(hxT, blk)).annotate(f'mul{blk}')
            pe_transpose(trh[blk // 2], blk % 2, rhT, blk)
            cast_fp8(nc.gpsimd, candpack[:, 0, blk * BLK:(blk + 1) * BLK],
                     nb(rhT, blk), S_H).annotate(f'cp0_{blk}')
            eng = (nc.scalar, nc.scalar, nc.vector, nc.vector)[blk]
            cast_fp8(eng, s8rh[:, 4 * blk:4 * (blk + 1), :],
                     trh[blk // 2][:, (blk % 2) * BLK:(blk % 2 + 1) * BLK],
                     S_H).annotate(f'evR{blk}')
        # --- phase B: mmB quarters; u gates + lrh casts in the shadow --
        psB = [None] * 4
        for q in range(4):
            psB[q] = big_ps.tile([128, BLK], F32, tag="big", name=f"psB{q}")
            big_mm_pairs(s8rh, psB[q], q, range(6), 0, NPAIR - 1)
        for q in range(4):
            big_mm_pairs(s8rh, psB[q], q, (6, 7), 0, NPAIR - 1)
            cast_fp8(nc.vector, candpack[:, 1, q * BLK:(q + 1) * BLK],
                     psB[q][:], SC_LXH).annotate(f'lrh{q}')
            if q >= 2:
                blk = q - 2
                alloc_u(blk)
                gate_dr(ups[blk], w8["w8_u"], rupack, blk)
                nc.scalar.activation(nb(uT, blk), ups[blk][:], AF.Sigmoid,
                                     scale=1.0 / G)
        for blk in (2, 3):
            alloc_u(blk)
            gate_dr(ups[blk], w8["w8_u"], rupack, blk)
            nc.scalar.activation(nb(uT, blk), ups[blk][:], AF.Sigmoid,
                                 scale=1.0 / G)
        for blk in range(NBLK):
            nc.gpsimd.tensor_mul(nb(WT, blk), nb(uT, blk),
                                 nb(hxT, blk)).annotate(f'W{blk}')
            nc.vector.tensor_scalar(nb(vT, blk), nb(uT, blk), -1.0, 1.0,
                                    op0=MUL, op1=ADD).annotate(f'v{blk}')
        # --- phase B2: cand -> tanh -> blend (hy = cy + u*(hx-cy)) -----
        candps = [None] * 4
        for q in range(4):
            cp = big_ps.tile([128, BLK], F32, tag="big", name=f"cand{q}")
            candps[q] = cp
            gate_ident(cp, gcc, q)
            gate_dr(cp, w8["w8_c"], candpack, q)
        for q in range(4):
            nc.scalar.activation(nb(cyT, q), candps[q][:], AF.Tanh,
                                 scale=1.0 / G).annotate(f'tanh{q}')
        thy = [t_ps.tile([128, 2 * BLK], BF16, tag="tr", name=f"thy{h}")
               for h in range(2)] if t < T - 1 else None

        def blend(blk):
            pp = work.tile([128, BLK], BF16, tag="tmp", name="pp")
            nc.vector.tensor_mul(pp[:], nb(vT, blk), nb(cyT, blk)).annotate(f'p{blk}')
            nc.vector.tensor_add(nb(hyT, blk), nb(WT, blk), pp[:]).annotate(f'hy{blk}')

        def hyexp(blk):
            if t < T - 1:
                pe_transpose(thy[blk // 2], blk % 2, hyT, blk)

        def hyevac(blk):
            if t >= T - 1:
                return
            eng = (nc.scalar, nc.vector, nc.scalar, nc.vector)[blk]
            cast_fp8(eng, s8hy[:, 4 * blk:4 * (blk + 1), :],
                     thy[blk // 2][:, (blk % 2) * BLK:(blk % 2 + 1) * BLK],
                     S_H).annotate(f'evH{blk}')

        for blk in range(NBLK):
            blend(blk)
            hyexp(blk)
            hyevac(blk)
        if t < T - 1:
            cast_fp8(nc.gpsimd, rupack[:, 0, :], hyT[:], S_H)
        with tc.high_priority(offset=-400):
            for blk in range(NBLK):
                ep = big_ps.tile([128, BLK], F32, tag="big", name=f"edge{blk}")
                nc.tensor.matmul(ep[:], wb["web"], nb(hyT, blk), start=True,
                                 stop=True, skip_group_check=True)
                nc.scalar.activation(
                    ytbuf[:, t * N + blk * BLK:t * N + (blk + 1) * BLK],
                    ep[:], AF.Sigmoid, bias=bias["bee"][:])
        if t >= 2:
            nc.sync.dma_start(d["out"][t - 2, :, :], ytbuf[:, (t - 2) * N:(t - 1) * N])
        if t == T - 1:
            # out[T-2] has been ready since the previous step's sigmoids;
            # out[T-1] goes per block, each DMA chasing its own sigmoid, so
            # the final transfer tail is one [128,512] DMA, not a full row.
            nc.sync.dma_start(d["out"][T - 2, :, :],
                              ytbuf[:, (T - 2) * N:(T - 1) * N])
            for blk in range(NBLK):
                nc.sync.dma_start(
                    d["out"][T - 1, :, blk * BLK:(blk + 1) * BLK],
                    ytbuf[:, (T - 1) * N + blk * BLK:(T - 1) * N + (blk + 1) * BLK])


_BUILT = {}


def _build():
    if "nc" in _BUILT:
        return _BUILT["nc"]
    nc = bacc.Bacc("TRN2", target_bir_lowering=False, debug=False)
    d = {}
    d["lt8"] = nc.dram_tensor("lt8", [4, N, BLK], FP8,
                              kind="ExternalInput").ap()
    d["xnat8"] = nc.dram_tensor("xnat8", [N, C], FP8, kind="ExternalInput").ap()
    d["xT"] = nc.dram_tensor("xT", [C, N], BF16, kind="ExternalInput").ap()
    d["wpack8"] = nc.dram_tensor("wpack8", [128, 6 * 128], FP8,
                                 kind="ExternalInput").ap()
    d["wpackb"] = nc.dram_tensor("wpackb", [128, len(WB_NAMES) * 128], BF16,
                                 kind="ExternalInput").ap()
    d["bpack"] = nc.dram_tensor("bpack", [128, len(B_NAMES)], F32,
                                kind="ExternalInput").ap()
    d["out"] = nc.dram_tensor("out", [T, C, N], BF16, kind="ExternalOutput").ap()

    with tile.TileContext(nc) as tc, ExitStack() as ctx:
        _emit(ctx, tc, d)
    nc.compile()
    _BUILT["nc"] = nc
    return nc


def _bd(m):
    """[64,64] -> block-diagonal [128,128] (two independent batches)."""
    z = np.zeros((128, 128), np.float32)
    z[:64, :64] = m
    z[64:, 64:] = m
    return z


def _q8(a):
    import ml_dtypes
    e4 = getattr(ml_dtypes, "float8_e4m3fn", None) or ml_dtypes.float8_e4m3
    return np.clip(np.asarray(a, np.float32), -240.0, 240.0).astype(e4)


def _bf(a):
    import ml_dtypes
    return np.asarray(a, np.float32).astype(ml_dtypes.bfloat16)


def make_in_maps(inputs_edge, L_tilde, W_gate, b_gate, W_upd, b_upd,
                 W_edge, b_edge):
    """Host-side layout transforms + quantization + per-core sharding."""
    x = np.asarray(inputs_edge, np.float32)
    L = np.asarray(L_tilde, np.float32)
    Wg0, Wg1 = np.asarray(W_gate[0], np.float32), np.asarray(W_gate[1], np.float32)
    Wu0, Wu1 = np.asarray(W_upd[0], np.float32), np.asarray(W_upd[1], np.float32)
    We = np.asarray(W_edge, np.float32)
    bg = np.asarray(b_gate, np.float32)
    bu = np.asarray(b_upd, np.float32)
    be = np.asarray(b_edge, np.float32)

    # fp8 DR weight pairs, scales folded:
    #   slot0 (vs hxT8 = S_H*hx):   (G/S_H) * wh0
    #   slot1 (vs lxh8 = S_C*lxh):  (G/S_C) * wh1
    s0, s1 = G / S_H, G / S_C
    wpack8 = np.concatenate([
        _bd(s0 * Wg0[F:, :F]), _bd(s1 * Wg1[F:, :F]),      # r
        _bd(s0 * Wg0[F:, F:]), _bd(s1 * Wg1[F:, F:]),      # u
        _bd(s0 * Wu0[F:]), _bd(s1 * Wu1[F:]),              # cand
    ], axis=1)
    wpackb = np.concatenate([
        np.eye(128, dtype=np.float32), _bd(We),
        _bd(G * Wg0[:F, :F]), _bd(G * Wg1[:F, :F]),
        _bd(G * Wg0[:F, F:]), _bd(G * Wg1[:F, F:]),
        _bd(G * Wu0[:F]), _bd(G * Wu1[:F]),
    ], axis=1)
    bpack = np.stack([G * np.tile(bg[:F], 2), G * np.tile(bg[F:], 2),
                      G * np.tile(bu, 2), np.tile(be, 2)], axis=1)
    shared = {
        "lt8": np.ascontiguousarray(
            _q8(S_L * L.T).reshape(N, 4, BLK).transpose(1, 0, 2)),
        "wpack8": np.ascontiguousarray(_q8(wpack8)),
        "wpackb": np.ascontiguousarray(_bf(wpackb)),
        "bpack": np.ascontiguousarray(bpack.astype(np.float32)),
    }
    in_maps = []
    for core in range(NCORES):
        xs = x[core * BL:(core + 1) * BL]                    # [BL, N, F]
        m = dict(shared)
        m["xnat8"] = np.ascontiguousarray(
            _q8(S_H * xs.transpose(1, 0, 2).reshape(N, C)))
        m["xT"] = np.ascontiguousarray(
            _bf(xs.transpose(0, 2, 1).reshape(C, N)))
        in_maps.append(m)
    return in_maps


def unshard(core_outs):
    """[NCORES][T, C, N] (bf16) -> [T, B, N, F] fp32"""
    arr = np.stack([np.asarray(o, np.float32) for o in core_outs])
    return np.ascontiguousarray(
        arr.reshape(NCORES, T, BL, F, N)
           .transpose(1, 0, 2, 4, 3)
           .reshape(T, B, N, F).astype(np.float32))


def run(in_maps, **kw):
    nc = _build()
    return run_bass_kernel_spmd(nc, in_maps, list(range(NCORES)), **kw)


def kernel(inputs_edge, L_tilde, W_gate, b_gate, W_upd, b_upd, W_edge, b_edge):
    in_maps = make_in_maps(inputs_edge, L_tilde, W_gate, b_gate,
                           W_upd, b_upd, W_edge, b_edge)
    res = run(in_maps)
    return unshard([res.results[c]["out"] for c in range(NCORES)])


# revision 60
# speedup vs baseline: 1.0031x; 1.0030x over previous
# Trainium2 Bass kernel for the Chebyshev-GCN GRU decoder (gnn_message_passing).
#
# Problem: B=16, N=2048, F=64, K=2 Chebyshev taps, T=8 decode steps.
#   per step: gates = cheb(L, [x, hx]) @ W_gate; r,u = sigmoid(gates)
#             cy = tanh(cheb(L, [x, r*hx]) @ W_upd); hy = u*hx + (1-u)*cy
#             yt = sigmoid(hy @ W_edge)
#
# Strategy (168.2us baseline -> 141.7us; HW rel err 1.32e-2):
#  - Data-parallel over batch: 8 cores x 2 batches each; c = b*64+f = 128
#    partitions for all "transposed"-layout [c, n] tensors.
#  - Big matmuls (L@hx, L@(r*hx)) are fp8e4m3 DoubleRow: stationary = fp8
#    state m-tile pairs [128, 2, 128], moving = fp8 L^T [128, 2, 256]; one
#    instruction contracts K=256 at 0.5 cycles/row.  L is pre-scaled x64 on
#    host, state x16 on device; scales fold into the fp8 gate weights.
#  - Each big matmul runs as FOUR quarter-psum passes (all 8 K-pairs x 512
#    cols, one bank each, one 512-wide DR instruction per K-pair; `start`
#    zeroes a whole psum bank).  The big ring holds 4 banks so
#    all quarters fly at once; mmB additionally emits pairs 0-5 of every
#    quarter before any pair-6/7 tail so the in-order PE queue never blocks
#    early quarters on the last-arriving stationary block.
#  - Natural-layout fp8 stationaries come from PE transposes (bf16 in,
#    bf16 psum out, 53ns per 128x128 tile) instead of DMA XBAR: the DMA
#    path cost 625 (queue) + 448 (xfer) + 900 (sem) per block on the
#    recurrence chain; the PE path is 212 + evac-cast + 100 sem.  The
#    psum->SBUF evac casts double as the x16 fp8 quantization.
#  - r-gate DRs interleave between mmA quarter passes so the sigma_r
#    cascade hides under mmA; r2/r3 psums borrow big-ring banks so the
#    2-bank r ring (shared with the u gates) never gates the cascade.
#    The u path (sigma_u, W = u*hx on Pool, v = 1-u) rides in mmB's
#    shadow; blend is pp = v*cy, hy = W + pp (2 DVE hops after each tanh).
#  - PSUM (8 banks): big ring 4x[128,512] f32 (psA/psB quarters, r2/r3,
#    cand, edge); r/u ring 2x[128,512]; transpose ring 2x[128,1024] bf16.
#  - Pre-roll: lt8 is loaded in column-quarter DMAs (SBUF [128,4,NT,512])
#    ordered xnat8 -> lt8q0 -> xT/wpackb/bpack -> lt8q1..3 -> wpack8, so
#    the precompute big-matmul quarters, the gate constants gc{r,u,c}
#    (DVE stt + bias, emitted u/c before r since step 0 only needs u,c)
#    and step 0 pipeline behind the 11.6us L load.
#  - Output sigmoids land in an SBUF ring, DMA'd out two steps later.
#
# kernel() takes FULL unsharded inputs, returns FULL [T, B, N, F] fp32.

import numpy as np
from contextlib import ExitStack

import concourse.bass as bass
import concourse.tile as tile
from concourse import bacc, mybir
from concourse.bass_utils import run_bass_kernel_spmd

F32 = mybir.dt.float32
BF16 = mybir.dt.bfloat16
FP8 = mybir.dt.float8e4
DR = mybir.MatmulPerfMode.DoubleRow

B, N, F = 16, 2048, 64
T = 8
NCORES = 8
BL = B // NCORES          # batches per core (2)
C = BL * F                # 128 partitions
NT = N // 128             # 16 m-tiles
NBLK = 4                  # n blocks for elementwise/small-mm work
BLK = N // NBLK           # 512
NPAIR = NT // 2           # 8 DoubleRow K-pairs

S_L = 64.0                # L^T fp8 pre-scale (host)
S_H = 16.0                # hx / rh fp8 cast scale (device)
S_C = 4.0                 # lxh8 scale; psum A holds S_L*S_H*lxh
G = 64.0                  # gate/cand PSUM pre-activation scale

W8_PAIRS = ["w8_r", "w8_u", "w8_c"]
WB_NAMES = ["identb", "web", "wx0r", "wx1r", "wx0u", "wx1u", "wxc0", "wxc1"]
B_NAMES = ["bgr", "bgu", "bcc", "bee"]

MUL = mybir.AluOpType.mult
ADD = mybir.AluOpType.add
SUBTRACT = mybir.AluOpType.subtract


def _emit(ctx: ExitStack, tc: tile.TileContext, d):
    nc = tc.nc
    AF = mybir.ActivationFunctionType

    consts = ctx.enter_context(tc.tile_pool(name="consts", bufs=1))
    work = ctx.enter_context(tc.tile_pool(name="work", bufs=3))
    # PSUM (8 banks): big ring 4x[128,512] = 4 banks (all 4 quarter passes of
    # a big matmul can be in flight); r/u shared ring 2x[128,512] (u-gate
    # psums reuse the r banks once sigma_r consumed them); transpose ring
    # 2x[128,1024] bf16 = 2 banks.
    big_ps = ctx.enter_context(tc.tile_pool(name="bigps", bufs=4, space="PSUM"))
    r_ps = ctx.enter_context(tc.tile_pool(name="rps", bufs=2, space="PSUM"))
    u_ps = r_ps
    t_ps = ctx.enter_context(tc.tile_pool(name="tps", bufs=2, space="PSUM"))

    # ---- static loads -------------------------------------------------
    # Order matters: the single DMA queue serializes transfers, and the
    # precompute big-matmul quarter q is gated by xnat8 + lt8 quarter q.
    xnat8 = consts.tile([128, NT, 128], FP8, tag="xnat8")
    try:
        nc.sync.dma_start(xnat8[:], d["xnat8"].rearrange("(a p) c -> p a c", p=128))
    except Exception:
        for mi in range(NT):
            nc.sync.dma_start(xnat8[:, mi, :],
                              d["xnat8"][mi * 128:(mi + 1) * 128, :])
    lt8 = consts.tile([128, 4, NT, BLK], FP8, tag="lt8")
    nc.sync.dma_start(lt8[:, 0, :, :],
                      d["lt8"][0].rearrange("(a p) c -> p a c", p=128))
    xT = consts.tile([128, N], BF16, tag="xT")
    nc.sync.dma_start(xT[:], d["xT"][:, :])
    wpackb = consts.tile([128, len(WB_NAMES) * 128], BF16, tag="wpackb")
    nc.sync.dma_start(wpackb[:], d["wpackb"][:, :])
    wb = {name: wpackb[:, i * 128:(i + 1) * 128]
          for i, name in enumerate(WB_NAMES)}
    bpack = consts.tile([128, len(B_NAMES)], F32, tag="bpack")
    nc.sync.dma_start(bpack[:], d["bpack"][:, :])
    bias = {name: bpack[:, j:j + 1] for j, name in enumerate(B_NAMES)}
    for q in range(1, 4):
        nc.sync.dma_start(lt8[:, q, :, :],
                          d["lt8"][q].rearrange("(a p) c -> p a c", p=128))
    wpack8 = consts.tile([128, 6, 128], FP8, tag="wpack8")
    nc.sync.dma_start(wpack8[:], d["wpack8"][:, :].rearrange("p (i m) -> p i m", i=6))
    w8 = {name: wpack8[:, 2 * i:2 * i + 2, :] for i, name in enumerate(W8_PAIRS)}

    # ---- persistent step buffers --------------------------------------
    hxbuf = [consts.tile([128, N], BF16, tag=f"hxT{i}", name=f"hxT{i}")
             for i in range(2)]
    ruT = consts.tile([128, 2, N], BF16, tag="ruT")   # r plane | u plane
    cyT = consts.tile([128, N], BF16, tag="cyT")
    rhT = consts.tile([128, N], BF16, tag="rhT")
    s8hy = consts.tile([128, NT, 128], FP8, tag="s8hy")
    s8rh = consts.tile([128, NT, 128], FP8, tag="s8rh")
    rupack = consts.tile([128, 2, N], FP8, tag="rupack")      # hxT8 | lxh8
    candpack = consts.tile([128, 2, N], FP8, tag="candpack")  # rhT8 | lrh8
    WT = consts.tile([128, N], BF16, tag="WT")
    vT = consts.tile([128, N], BF16, tag="vT")
    gcr = consts.tile([128, N], BF16, tag="gcr")
    gcu = consts.tile([128, N], BF16, tag="gcu")
    gcc = consts.tile([128, N], BF16, tag="gcc")
    LxT = consts.tile([128, N], BF16, tag="LxT")
    ytbuf = consts.tile([128, T * N], BF16, tag="ytbuf")

    def nb(ap, blk):
        return ap[:, blk * BLK:(blk + 1) * BLK]

    def big_mm_quarter(stat8, ps, q):
        """One psum quarter (cols [512q, 512q+512)) of the DoubleRow big
        matmul: all 8 K-pairs, 2x256-col instructions each."""
        for p in range(NPAIR):
            nc.tensor.matmul(
                ps[:], stat8[:, 2 * p:2 * p + 2, :],
                lt8[:, q, 2 * p:2 * p + 2, :],
                start=(p == 0), stop=(p == NPAIR - 1),
                perf_mode=DR, skip_group_check=True)

    def big_mm_pairs(stat8, ps, q, pairs, start_p, stop_p):
        for p in pairs:
            nc.tensor.matmul(
                ps[:], stat8[:, 2 * p:2 * p + 2, :],
                lt8[:, q, 2 * p:2 * p + 2, :],
                start=(p == start_p), stop=(p == stop_p),
                perf_mode=DR, skip_group_check=True)

    def gate_ident(ps, gc, blk):
        nc.tensor.matmul(ps[:], wb["identb"], nb(gc, blk),
                         start=True, stop=False, skip_group_check=True)

    def gate_dr(ps, wpair, pack, blk):
        nc.tensor.matmul(
            ps[:], wpair, pack[:, :, blk * BLK:(blk + 1) * BLK],
            start=False, stop=True,
            perf_mode=DR, skip_group_check=True)

    def pe_transpose(tr, half, src_row, blk):
        """Transpose block blk of bf16 row src_row into tr cols
        [512*half, 512*half+512) as 4 PE 128x128 transposes."""
        for j in range(4):
            nc.tensor.matmul(
                tr[:, half * BLK + j * 128:half * BLK + (j + 1) * 128],
                src_row[:, blk * BLK + j * 128:blk * BLK + (j + 1) * 128],
                wb["identb"], is_transpose=True,
                skip_group_check=True).annotate(f'T{blk}_{j}')

    def cast_fp8(eng, dst, src, scale):
        if eng is nc.scalar:
            return eng.mul(dst, src, float(scale))
        return eng.tensor_scalar_mul(dst, src, float(scale))

    SC_LXH = S_C / (S_L * S_H)

    # =========== precompute ============================================
    for q in range(4):
        pq = big_ps.tile([128, BLK], F32, tag="big", name=f"pre{q}")
        big_mm_quarter(xnat8, pq, q)
        cast_fp8(nc.scalar, LxT[:, q * BLK:(q + 1) * BLK], pq[:],
                 1.0 / (S_L * S_H))
    for blk in range(NBLK):
        for pool, tg, wa, wc_, dst, bs in (
                (u_ps, "r", "wx0u", "wx1u", gcu, "bgu"),
                (r_ps, "r", "wxc0", "wxc1", gcc, "bcc"),
                (r_ps, "r", "wx0r", "wx1r", gcr, "bgr")):
            ps = pool.tile([128, BLK], F32, tag=tg, name="cps")
            nc.tensor.matmul(ps[:], wb[wa], nb(xT, blk), start=True, stop=False,
                             skip_group_check=True)
            nc.tensor.matmul(ps[:], wb[wc_], nb(LxT, blk), start=False, stop=True,
                             skip_group_check=True)
            nc.vector.scalar_tensor_tensor(
                nb(dst, blk), ps[:], 1.0,
                bias[bs][:].broadcast_to([128, BLK]), op0=MUL, op1=ADD)

    # =========== step 0 (hx == 0) ======================================
    hyT = hxbuf[1]
    uT0 = ruT[:, 1, :]
    for blk in range(NBLK):
        nc.scalar.activation(nb(uT0, blk), nb(gcu, blk), AF.Sigmoid,
                             scale=1.0 / G)
        nc.scalar.activation(nb(cyT, blk), nb(gcc, blk), AF.Tanh,
                             scale=1.0 / G)
        e = work.tile([128, BLK], BF16, tag="tmp", name="e0")
        nc.vector.tensor_mul(e[:], nb(uT0, blk), nb(cyT, blk))
        nc.vector.tensor_sub(nb(hyT, blk), nb(cyT, blk), e[:])
    tr0 = [t_ps.tile([128, 2 * BLK], BF16, tag="tr", name=f"tr0{h}")
           for h in range(2)]
    for blk in range(NBLK):
        pe_transpose(tr0[blk // 2], blk % 2, hyT, blk)
    for blk in range(NBLK):
        eng = (nc.vector, nc.scalar, nc.vector, nc.scalar)[blk]
        cast_fp8(eng, s8hy[:, 4 * blk:4 * (blk + 1), :],
                 tr0[blk // 2][:, (blk % 2) * BLK:(blk % 2 + 1) * BLK], S_H)
    for blk in range(NBLK):
        cast_fp8(nc.gpsimd, rupack[:, 0, blk * BLK:(blk + 1) * BLK],
                 nb(hyT, blk), S_H)
    with tc.high_priority(offset=-400):
        for blk in range(NBLK):
            ep = big_ps.tile([128, BLK], F32, tag="big", name=f"edg0{blk}")
            nc.tensor.matmul(ep[:], wb["web"], nb(hyT, blk), start=True,
                             stop=True, skip_group_check=True)
            nc.scalar.activation(ytbuf[:, blk * BLK:(blk + 1) * BLK], ep[:],
                                 AF.Sigmoid, bias=bias["bee"][:])

    # =========== steps 1..T-1 ==========================================
    # Per step, the critical chain is:
    #   mmA q -> lxh cast -> rDR -> sigma_r -> rh mul -> PE transpose ->
    #   s8rh evac-cast -> mmB -> lrh cast -> candDR -> tanh -> d/m/hy ->
    #   PE transpose -> s8hy evac-cast -> mmA' ...
    # r-gate DRs are interleaved between mmA quarter passes so the sigma_r
    # cascade hides under mmA; r2/r3 psums come from the big ring so the
    # r ring (which also serves the u gates) never gates the cascade.
    rps = [None] * NBLK
    ups = [None] * NBLK

    def alloc_r(blk, pool):
        rps[blk] = pool.tile([128, BLK], F32, tag=("r" if pool is r_ps else "big"),
                             name="rps")
        gate_ident(rps[blk], gcr, blk)

    def alloc_u(blk):
        ups[blk] = u_ps.tile([128, BLK], F32, tag="r", name="ups")
        gate_ident(ups[blk], gcu, blk)

    rT = ruT[:, 0, :]
    uT = ruT[:, 1, :]

    for t in range(1, T):
        hxT, hyT = hxbuf[t % 2], hxbuf[(t + 1) % 2]
        # --- phase A: mmA quarters with rDRs interleaved ---------------
        psA = [None] * 4
        for q in range(4):
            psA[q] = big_ps.tile([128, BLK], F32, tag="big", name=f"psA{q}")
            big_mm_quarter(s8hy, psA[q], q)
            cast_fp8(nc.scalar if q == 1 else nc.vector,
                     rupack[:, 1, q * BLK:(q + 1) * BLK],
                     psA[q][:], SC_LXH).annotate(f'lxh{q}')
            if q == 0:
                alloc_r(0, r_ps)
                alloc_r(1, r_ps)
            if q in (1, 2):
                gate_dr(rps[q - 1], w8["w8_r"], rupack, q - 1)
                nc.scalar.activation(nb(rT, q - 1), rps[q - 1][:], AF.Sigmoid,
                                     scale=1.0 / G).annotate(f'sigr{q - 1}')
            elif q == 3:
                alloc_r(2, big_ps)
                gate_dr(rps[2], w8["w8_r"], rupack, 2)
                nc.scalar.activation(nb(rT, 2), rps[2][:], AF.Sigmoid,
                                     scale=1.0 / G).annotate('sigr2')
        alloc_r(3, big_ps)
        gate_dr(rps[3], w8["w8_r"], rupack, 3)
        nc.scalar.activation(nb(rT, 3), rps[3][:], AF.Sigmoid,
                             scale=1.0 / G).annotate('sigr3')
        # rh mul -> PE transpose -> evac (Act for blks 0/1, DVE for 2/3)
        trh = [t_ps.tile([128, 2 * BLK], BF16, tag="tr", name=f"trh{h}")
               for h in range(2)]
        for blk in range(NBLK):
            nc.vector.tensor_mul(nb(rhT, blk), nb(rT, blk),
                                 nb(hxT, blk)).annotate(f'mul{blk}')
            pe_transpose(trh[blk // 2], blk % 2, rhT, blk)
            cast_fp8(nc.gpsimd, candpack[:, 0, blk * BLK:(blk + 1) * BLK],
                     nb(rhT, blk), S_H).annotate(f'cp0_{blk}')
            eng = (nc.scalar, nc.scalar, nc.vector, nc.vector)[blk]
            cast_fp8(eng, s8rh[:, 4 * blk:4 * (blk + 1), :],
                     trh[blk // 2][:, (blk % 2) * BLK:(blk % 2 + 1) * BLK],
                     S_H).annotate(f'evR{blk}')
        # --- phase B: mmB quarters; u gates + lrh casts in the shadow --
        psB = [None] * 4
        for q in range(4):
            psB[q] = big_ps.tile([128, BLK], F32, tag="big", name=f"psB{q}")
            big_mm_pairs(s8rh, psB[q], q, range(6), 0, NPAIR - 1)
        for q in range(4):
            big_mm_pairs(s8rh, psB[q], q, (6, 7), 0, NPAIR - 1)
            cast_fp8(nc.vector, candpack[:, 1, q * BLK:(q + 1) * BLK],
                     psB[q][:], SC_LXH).annotate(f'lrh{q}')
            if q >= 2:
                blk = q - 2
                alloc_u(blk)
                gate_dr(ups[blk], w8["w8_u"], rupack, blk)
                nc.scalar.activation(nb(uT, blk), ups[blk][:], AF.Sigmoid,
                                     scale=1.0 / G)
        for blk in (2, 3):
            alloc_u(blk)
            gate_dr(ups[blk], w8["w8_u"], rupack, blk)
            nc.scalar.activation(nb(uT, blk), ups[blk][:], AF.Sigmoid,
                                 scale=1.0 / G)
        for blk in range(NBLK):
            nc.gpsimd.tensor_mul(nb(WT, blk), nb(uT, blk),
                                 nb(hxT, blk)).annotate(f'W{blk}')
            nc.vector.tensor_scalar(nb(vT, blk), nb(uT, blk), -1.0, 1.0,
                                    op0=MUL, op1=ADD).annotate(f'v{blk}')
        # --- phase B2: cand -> tanh -> blend (hy = cy + u*(hx-cy)) -----
        candps = [None] * 4
        for q in range(4):
            cp = big_ps.tile([128, BLK], F32, tag="big", name=f"cand{q}")
            candps[q] = cp
            gate_ident(cp, gcc, q)
            gate_dr(cp, w8["w8_c"], candpack, q)
        for q in range(4):
            nc.scalar.activation(nb(cyT, q), candps[q][:], AF.Tanh,
                                 scale=1.0 / G).annotate(f'tanh{q}')
        thy = [t_ps.tile([128, 2 * BLK], BF16, tag="tr", name=f"thy{h}")
               for h in range(2)] if t < T - 1 else None

        def blend(blk):
            pp = work.tile([128, BLK], BF16, tag="tmp", name="pp")
            nc.vector.tensor_mul(pp[:], nb(vT, blk), nb(cyT, blk)).annotate(f'p{blk}')
            nc.vector.tensor_add(nb(hyT, blk), nb(WT, blk), pp[:]).annotate(f'hy{blk}')

        def hyexp(blk):
            if t < T - 1:
                pe_transpose(thy[blk // 2], blk % 2, hyT, blk)

        def hyevac(blk):
            if t >= T - 1:
                return
            eng = (nc.scalar, nc.scalar, nc.vector, nc.vector)[blk]
            cast_fp8(eng, s8hy[:, 4 * blk:4 * (blk + 1), :],
                     thy[blk // 2][:, (blk % 2) * BLK:(blk % 2 + 1) * BLK],
                     S_H).annotate(f'evH{blk}')

        for blk in range(NBLK):
            blend(blk)
            hyexp(blk)
            hyevac(blk)
        if t < T - 1:
            cast_fp8(nc.gpsimd, rupack[:, 0, :], hyT[:], S_H)
        with tc.high_priority(offset=-400):
            for blk in range(NBLK):
                ep = big_ps.tile([128, BLK], F32, tag="big", name=f"edge{blk}")
                nc.tensor.matmul(ep[:], wb["web"], nb(hyT, blk), start=True,
                                 stop=True, skip_group_check=True)
                nc.scalar.activation(
                    ytbuf[:, t * N + blk * BLK:t * N + (blk + 1) * BLK],
                    ep[:], AF.Sigmoid, bias=bias["bee"][:])
        if t >= 2:
            nc.sync.dma_start(d["out"][t - 2, :, :], ytbuf[:, (t - 2) * N:(t - 1) * N])
        if t == T - 1:
            # out[T-2] has been ready since the previous step's sigmoids;
            # out[T-1] goes per block, each DMA chasing its own sigmoid, so
            # the final transfer tail is one [128,512] DMA, not a full row.
            nc.sync.dma_start(d["out"][T - 2, :, :],
                              ytbuf[:, (T - 2) * N:(T - 1) * N])
            for blk in range(NBLK):
                nc.sync.dma_start(
                    d["out"][T - 1, :, blk * BLK:(blk + 1) * BLK],
                    ytbuf[:, (T - 1) * N + blk * BLK:(T - 1) * N + (blk + 1) * BLK])


_BUILT = {}


def _build():
    if "nc" in _BUILT:
        return _BUILT["nc"]
    nc = bacc.Bacc("TRN2", target_bir_lowering=False, debug=False)
    d = {}
    d["lt8"] = nc.dram_tensor("lt8", [4, N, BLK], FP8,
                              kind="ExternalInput").ap()
    d["xnat8"] = nc.dram_tensor("xnat8", [N, C], FP8, kind="ExternalInput").ap()
    d["xT"] = nc.dram_tensor("xT", [C, N], BF16, kind="ExternalInput").ap()
    d["wpack8"] = nc.dram_tensor("wpack8", [128, 6 * 128], FP8,
                                 kind="ExternalInput").ap()
    d["wpackb"] = nc.dram_tensor("wpackb", [128, len(WB_NAMES) * 128], BF16,
                                 kind="ExternalInput").ap()
    d["bpack"] = nc.dram_tensor("bpack", [128, len(B_NAMES)], F32,
                                kind="ExternalInput").ap()
    d["out"] = nc.dram_tensor("out", [T, C, N], BF16, kind="ExternalOutput").ap()

    with tile.TileContext(nc) as tc, ExitStack() as ctx:
        _emit(ctx, tc, d)
    nc.compile()
    _BUILT["nc"] = nc
    return nc


def _bd(m):
    """[64,64] -> block-diagonal [128,128] (two independent batches)."""
    z = np.zeros((128, 128), np.float32)
    z[:64, :64] = m
    z[64:, 64:] = m
    return z


def _q8(a):
    import ml_dtypes
    e4 = getattr(ml_dtypes, "float8_e4m3fn", None) or ml_dtypes.float8_e4m3
    return np.clip(np.asarray(a, np.float32), -240.0, 240.0).astype(e4)


def _bf(a):
    import ml_dtypes
    return np.asarray(a, np.float32).astype(ml_dtypes.bfloat16)


def make_in_maps(inputs_edge, L_tilde, W_gate, b_gate, W_upd, b_upd,
                 W_edge, b_edge):
    """Host-side layout transforms + quantization + per-core sharding."""
    x = np.asarray(inputs_edge, np.float32)
    L = np.asarray(L_tilde, np.float32)
    Wg0, Wg1 = np.asarray(W_gate[0], np.float32), np.asarray(W_gate[1], np.float32)
    Wu0, Wu1 = np.asarray(W_upd[0], np.float32), np.asarray(W_upd[1], np.float32)
    We = np.asarray(W_edge, np.float32)
    bg = np.asarray(b_gate, np.float32)
    bu = np.asarray(b_upd, np.float32)
    be = np.asarray(b_edge, np.float32)

    # fp8 DR weight pairs, scales folded:
    #   slot0 (vs hxT8 = S_H*hx):   (G/S_H) * wh0
    #   slot1 (vs lxh8 = S_C*lxh):  (G/S_C) * wh1
    s0, s1 = G / S_H, G / S_C
    wpack8 = np.concatenate([
        _bd(s0 * Wg0[F:, :F]), _bd(s1 * Wg1[F:, :F]),      # r
        _bd(s0 * Wg0[F:, F:]), _bd(s1 * Wg1[F:, F:]),      # u
        _bd(s0 * Wu0[F:]), _bd(s1 * Wu1[F:]),              # cand
    ], axis=1)
    wpackb = np.concatenate([
        np.eye(128, dtype=np.float32), _bd(We),
        _bd(G * Wg0[:F, :F]), _bd(G * Wg1[:F, :F]),
        _bd(G * Wg0[:F, F:]), _bd(G * Wg1[:F, F:]),
        _bd(G * Wu0[:F]), _bd(G * Wu1[:F]),
    ], axis=1)
    bpack = np.stack([G * np.tile(bg[:F], 2), G * np.tile(bg[F:], 2),
                      G * np.tile(bu, 2), np.tile(be, 2)], axis=1)
    shared = {
        "lt8": np.ascontiguousarray(
            _q8(S_L * L.T).reshape(N, 4, BLK).transpose(1, 0, 2)),
        "wpack8": np.ascontiguousarray(_q8(wpack8)),
        "wpackb": np.ascontiguousarray(_bf(wpackb)),
        "bpack": np.ascontiguousarray(bpack.astype(np.float32)),
    }
    in_maps = []
    for core in range(NCORES):
        xs = x[core * BL:(core + 1) * BL]                    # [BL, N, F]
        m = dict(shared)
        m["xnat8"] = np.ascontiguousarray(
            _q8(S_H * xs.transpose(1, 0, 2).reshape(N, C)))
        m["xT"] = np.ascontiguousarray(
            _bf(xs.transpose(0, 2, 1).reshape(C, N)))
        in_maps.append(m)
    return in_maps


def unshard(core_outs):
    """[NCORES][T, C, N] (bf16) -> [T, B, N, F] fp32"""
    arr = np.stack([np.asarray(o, np.float32) for o in core_outs])
    return np.ascontiguousarray(
        arr.reshape(NCORES, T, BL, F, N)
           .transpose(1, 0, 2, 4, 3)
           .reshape(T, B, N, F).astype(np.float32))


def run(in_maps, **kw):
    nc = _build()
    return run_bass_kernel_spmd(nc, in_maps, list(range(NCORES)), **kw)


def kernel(inputs_edge, L_tilde, W_gate, b_gate, W_upd, b_upd, W_edge, b_edge):
    in_maps = make_in_maps(inputs_edge, L_tilde, W_gate, b_gate,
                           W_upd, b_upd, W_edge, b_edge)
    res = run(in_maps)
    return unshard([res.results[c]["out"] for c in range(NCORES)])
